# revision 1
# baseline (speedup 1.0000x reference)
"""Self-contained Trainium2 Bass kernel for nn_DbrxBlock_40492951667588.

DBRX block: LN1 -> GQA attention (RoPE, causal) -> residual+LN2 -> top-2/8 MoE.
8 NeuronCores, two SPMD launches:
  launch 1: token-parallel attention (core r owns batch-0 block r + batch-1
            block 7-r; causal kv sets balance to 1152 tokens/core).
  host:     router softmax/top-2 from device logits, capacity-padded dispatch.
  launch 2: expert-parallel MoE (core e owns expert e).
Matmuls run in float32r (TF32-like, ~1.5e-4 rel err); LN weights are folded
into adjacent matmul weights on the host (exact).
"""
import numpy as np
import concourse.bacc as bacc
import concourse.bass as bass
import concourse.mybir as mybir
import concourse.tile as tile
from concourse.bass_utils import run_bass_kernel_spmd

F32 = mybir.dt.float32
F32R = mybir.dt.float32r
AF = mybir.ActivationFunctionType

D = 2048
DT = D // 128          # 16 d-tiles
TKV = 1152             # kv tokens per core
NKT = TKV // 128       # 9 kv tiles
TQ = 256               # own q tokens
NH, KVH, HD = 16, 4, 128
NQB = 2
EPS = 1e-5
NEG = -30000.0

SCH = [(0, 384), (384, 384), (768, 384)]   # TKV chunks (psum-bank sized)


def bc_ap(ap, parts, n):
    """Partition-broadcast read AP: [parts, n] from a [1, n] row."""
    return bass.AP(tensor=ap.tensor, offset=ap.offset, ap=[[0, parts], [1, n]])


def build_attn(n_cores=8):
    nc = bacc.Bacc("TRN2", target_bir_lowering=False, debug=False,
                   num_devices=n_cores)
    xt = nc.dram_tensor("xt", [DT, 128, TKV], F32R, kind="ExternalInput").ap()
    wk = nc.dram_tensor("wk", [KVH, 128, DT, 128], F32R, kind="ExternalInput").ap()
    wv = nc.dram_tensor("wv", [128, DT, 512], F32R, kind="ExternalInput").ap()
    wq = nc.dram_tensor("wq", [NH, 128, DT, 128], F32R, kind="ExternalInput").ap()
    wo = nc.dram_tensor("wo", [DT, 128, DT, 128], F32R, kind="ExternalInput").ap()
    wr = nc.dram_tensor("wr", [128, DT, 8], F32R, kind="ExternalInput").ap()
    wksum = nc.dram_tensor("wksum", [128, KVH], F32, kind="ExternalInput").ap()
    wqsum = nc.dram_tensor("wqsum", [128, NH], F32, kind="ExternalInput").ap()
    wvsum = nc.dram_tensor("wvsum", [1, 512], F32, kind="ExternalInput").ap()
    cosk = nc.dram_tensor("cosk", [128, TKV], F32, kind="ExternalInput").ap()
    sink = nc.dram_tensor("sink", [128, TKV], F32, kind="ExternalInput").ap()
    cosq = nc.dram_tensor("cosq", [128, TQ], F32, kind="ExternalInput").ap()
    sinq = nc.dram_tensor("sinq", [128, TQ], F32, kind="ExternalInput").ap()
    masks = nc.dram_tensor("masks", [NQB, 128, TKV], F32, kind="ExternalInput").ap()
    ones = nc.dram_tensor("ones", [128, 1], F32R, kind="ExternalInput").ap()
    ident = nc.dram_tensor("ident", [128, 128], F32R, kind="ExternalInput").ap()

    rest = nc.dram_tensor("rest", [DT, 128, TQ], F32, kind="ExternalOutput").ap()
    h2t = nc.dram_tensor("h2t", [DT, 128, TQ], F32, kind="ExternalOutput").ap()
    logt = nc.dram_tensor("logt", [8, TQ], F32, kind="ExternalOutput").ap()

    scratch = nc.dram_tensor("scratch", [4, TKV], F32).ap()  # stat-row bounce

    with tile.TileContext(nc) as tc:
        with (
            tc.tile_pool(name="rows", bufs=1) as rows,
            tc.tile_pool(name="kvq", bufs=1) as kvq,
        ):
            ones_sb = rows.tile([128, 1], F32R)
            nc.sync.dma_start(out=ones_sb[:], in_=ones[:])
            ident_sb = rows.tile([128, 128], F32R)
            nc.sync.dma_start(out=ident_sb[:], in_=ident[:])
            wksum_sb = rows.tile([128, KVH], F32)
            nc.sync.dma_start(out=wksum_sb[:], in_=wksum[:])
            wqsum_sb = rows.tile([128, NH], F32)
            nc.sync.dma_start(out=wqsum_sb[:], in_=wqsum[:])
            wvsum_bc = rows.tile([128, 512], F32)
            nc.sync.dma_start(out=wvsum_bc[:], in_=bc_ap(wvsum, 128, 512))
            eps_t = rows.tile([1, 1], F32)
            nc.vector.memset(eps_t[:], EPS)

            kT = kvq.tile([128, KVH, TKV], F32R)
            vN = kvq.tile([128, NKT, 512], F32R)
            qT = kvq.tile([128, NH, TQ], F32R)
            xq_res = kvq.tile([128, DT, TQ], F32)

            with tc.tile_pool(name="norm", bufs=1) as norm:
                rstd_bc = norm.tile([128, TKV], F32)
                nmr_bc = norm.tile([128, TKV], F32)
                rstd_col = norm.tile([128, NKT], F32)
                nmr_col = norm.tile([128, NKT], F32)

                with tc.tile_pool(name="xp", bufs=1) as xp:
                    xts = xp.tile([128, DT, TKV], F32R)
                    for d in range(DT):
                        nc.sync.dma_start(out=xts[:, d, :], in_=xt[d])
                    xtf = xts[:].bitcast(F32)

                    # ---------------- LN1 stats ----------------
                    with (
                        tc.tile_pool(name="strow", bufs=1) as strow,
                        tc.tile_pool(name="sqp", bufs=2) as sqp,
                        tc.tile_pool(name="ps_st", bufs=1, space="PSUM") as ps_st,
                    ):
                        mu_row = strow.tile([1, TKV], F32)
                        sqm_row = strow.tile([1, TKV], F32)
                        t_row = strow.tile([1, TKV], F32)
                        psum_s = [ps_st.tile([1, w], F32, name=f"pss{i}",
                                             tag=f"pss{i}")
                                  for i, (_, w) in enumerate(SCH)]
                        psum_q = [ps_st.tile([1, w], F32, name=f"psq{i}",
                                             tag=f"psq{i}")
                                  for i, (_, w) in enumerate(SCH)]
                        for d in range(DT):
                            sq = sqp.tile([128, TKV], F32R, tag="sq")
                            nc.scalar.activation(sq[:], xtf[:, d, :], AF.Square)
                            for i, (c0, w) in enumerate(SCH):
                                nc.tensor.matmul(psum_s[i][:], ones_sb[:],
                                                 xts[:, d, c0:c0 + w],
                                                 start=(d == 0),
                                                 stop=(d == DT - 1))
                                nc.tensor.matmul(psum_q[i][:], ones_sb[:],
                                                 sq[:, c0:c0 + w],
                                                 start=(d == 0),
                                                 stop=(d == DT - 1))
                        for i, (c0, w) in enumerate(SCH):
                            nc.scalar.mul(mu_row[:, c0:c0 + w], psum_s[i][:],
                                          1.0 / D)
                            nc.scalar.mul(sqm_row[:, c0:c0 + w], psum_q[i][:],
                                          1.0 / D)
                        # var = E[x^2]-mu^2; rstd=1/sqrt(var+eps); nmr=-mu*rstd
                        nc.vector.tensor_mul(t_row[:], mu_row[:], mu_row[:])
                        nc.vector.tensor_sub(sqm_row[:], sqm_row[:], t_row[:])
                        nc.scalar.activation(sqm_row[:], sqm_row[:], AF.Sqrt,
                                             bias=eps_t[:])
                        nc.vector.reciprocal(sqm_row[:], sqm_row[:])
                        nc.vector.tensor_mul(t_row[:], mu_row[:], sqm_row[:])
                        nc.scalar.mul(t_row[:], t_row[:], -1.0)
                        nc.sync.dma_start(out=scratch[0:1, :], in_=sqm_row[:])
                        nc.sync.dma_start(out=scratch[1:2, :], in_=t_row[:])
                        nc.sync.dma_start(out=rstd_bc[:],
                                          in_=bc_ap(scratch[0:1, :], 128, TKV))
                        nc.sync.dma_start(out=nmr_bc[:],
                                          in_=bc_ap(scratch[1:2, :], 128, TKV))
                        nc.sync.dma_start(
                            out=rstd_col[:],
                            in_=scratch[0, :].rearrange("(t p) -> p t", p=128))
                        nc.sync.dma_start(
                            out=nmr_col[:],
                            in_=scratch[1, :].rearrange("(t p) -> p t", p=128))

                    # ---------------- K proj + rope ----------------
                    with (
                        tc.tile_pool(name="ckp", bufs=1) as ckp,
                        tc.tile_pool(name="wkp", bufs=2) as wkp,
                        tc.tile_pool(name="ktp", bufs=2) as ktp,
                        tc.tile_pool(name="kf1", bufs=2) as kf1,
                        tc.tile_pool(name="ps_k", bufs=2, space="PSUM") as ps_k,
                    ):
                        cosk_sb = ckp.tile([128, TKV], F32)
                        nc.sync.dma_start(out=cosk_sb[:], in_=cosk[:])
                        sink_sb = ckp.tile([128, TKV], F32)
                        nc.sync.dma_start(out=sink_sb[:], in_=sink[:])
                        for ok in range(KVH):
                            wk_sb = wkp.tile([128, DT, 128], F32R, tag="wk")
                            nc.sync.dma_start(out=wk_sb[:], in_=wk[ok])
                            psk = [ps_k.tile([128, w], F32, name=f"psk{i}",
                                             tag=f"psk{i}")
                                   for i, (_, w) in enumerate(SCH)]
                            for d in range(DT):
                                for i, (c0, w) in enumerate(SCH):
                                    nc.tensor.matmul(psk[i][:], wk_sb[:, d, :],
                                                     xts[:, d, c0:c0 + w],
                                                     start=(d == 0),
                                                     stop=(d == DT - 1))
                            ktmp = ktp.tile([128, TKV], F32, tag="ktmp")
                            krot = ktp.tile([128, TKV], F32, tag="krot")
                            for i, (c0, w) in enumerate(SCH):
                                t1 = kf1.tile([128, 384], F32, tag="kpf1")
                                nc.scalar.activation(
                                    t1[:, :w], nmr_bc[:, c0:c0 + w], AF.Copy,
                                    scale=wksum_sb[:, ok:ok + 1])
                                nc.vector.tensor_mul(ktmp[:, c0:c0 + w],
                                                     psk[i][:],
                                                     rstd_bc[:, c0:c0 + w])
                                nc.vector.tensor_add(ktmp[:, c0:c0 + w],
                                                     ktmp[:, c0:c0 + w],
                                                     t1[:, :w])
                            nc.sync.dma_start(out=krot[0:64, :],
                                              in_=ktmp[64:128, :])
                            nc.sync.dma_start(out=krot[64:128, :],
                                              in_=ktmp[0:64, :])
                            nc.vector.tensor_mul(ktmp[:], ktmp[:], cosk_sb[:])
                            nc.vector.tensor_mul(krot[:], krot[:], sink_sb[:])
                            nc.vector.tensor_add(kT[:, ok, :], ktmp[:], krot[:])

                    # ---------------- V proj (t-major) ----------------
                    with (
                        tc.tile_pool(name="wvp", bufs=1) as wvp,
                        tc.tile_pool(name="vf1", bufs=2) as vf1,
                        tc.tile_pool(name="ps_v", bufs=2, space="PSUM") as ps_v,
                    ):
                        wv_sb = wvp.tile([128, DT, 512], F32R)
                        nc.sync.dma_start(out=wv_sb[:], in_=wv[:])
                        for tv in range(NKT):
                            psv = ps_v.tile([128, 512], F32, tag="psv")
                            for d in range(DT):
                                nc.tensor.matmul(
                                    psv[:], xts[:, d, tv * 128:(tv + 1) * 128],
                                    wv_sb[:, d, :],
                                    start=(d == 0), stop=(d == DT - 1))
                            t1 = vf1.tile([128, 512], F32, tag="vpf1")
                            nc.scalar.activation(t1[:], wvsum_bc[:], AF.Copy,
                                                 scale=nmr_col[:, tv:tv + 1])
                            t2 = vf1.tile([128, 512], F32, tag="vpf2")
                            nc.vector.tensor_scalar_mul(
                                t2[:], in0=psv[:],
                                scalar1=rstd_col[:, tv:tv + 1])
                            nc.vector.tensor_add(vN[:, tv, :], t1[:], t2[:])

                    # ---------------- Q proj + rope ----------------
                    with (
                        tc.tile_pool(name="cqp", bufs=1) as cqp,
                        tc.tile_pool(name="wqp", bufs=3) as wqp,
                        tc.tile_pool(name="qtp", bufs=2) as qtp,
                        tc.tile_pool(name="ps_q", bufs=2, space="PSUM") as ps_q,
                    ):
                        cosq_sb = cqp.tile([128, TQ], F32)
                        nc.sync.dma_start(out=cosq_sb[:], in_=cosq[:])
                        sinq_sb = cqp.tile([128, TQ], F32)
                        nc.sync.dma_start(out=sinq_sb[:], in_=sinq[:])
                        for oq in range(NH):
                            wq_sb = wqp.tile([128, DT, 128], F32R, tag="wq")
                            nc.sync.dma_start(out=wq_sb[:], in_=wq[oq])
                            psq = ps_q.tile([128, TQ], F32, tag="psq")
                            for d in range(DT):
                                nc.tensor.matmul(psq[:], wq_sb[:, d, :],
                                                 xts[:, d, 0:TQ],
                                                 start=(d == 0),
                                                 stop=(d == DT - 1))
                            qtmp = qtp.tile([128, TQ], F32, tag="qtmp")
                            qrot = qtp.tile([128, TQ], F32, tag="qrot")
                            t1 = qtp.tile([128, TQ], F32, tag="qpf1")
                            nc.scalar.activation(t1[:], nmr_bc[:, 0:TQ],
                                                 AF.Copy,
                                                 scale=wqsum_sb[:, oq:oq + 1])
                            nc.vector.tensor_mul(qtmp[:], psq[:],
                                                 rstd_bc[:, 0:TQ])
                            nc.vector.tensor_add(qtmp[:], qtmp[:], t1[:])
                            nc.sync.dma_start(out=qrot[0:64, :],
                                              in_=qtmp[64:128, :])
                            nc.sync.dma_start(out=qrot[64:128, :],
                                              in_=qtmp[0:64, :])
                            nc.vector.tensor_mul(qtmp[:], qtmp[:], cosq_sb[:])
                            nc.vector.tensor_mul(qrot[:], qrot[:], sinq_sb[:])
                            nc.vector.tensor_add(qT[:, oq, :], qtmp[:], qrot[:])

                    # own-q raw x for the residual add (outlives xts)
                    nc.vector.tensor_copy(xq_res[:], xtf[:, :, 0:TQ])

            # ---------------- attention ----------------
            with tc.tile_pool(name="attp", bufs=1) as attp:
                attnT = attp.tile([128, NH, TQ], F32R)
                with (
                    tc.tile_pool(name="mkp", bufs=1) as mkp,
                    tc.tile_pool(name="scp", bufs=2) as scp,
                    tc.tile_pool(name="srp", bufs=2) as srp,
                    tc.tile_pool(name="ptsp", bufs=2) as ptsp,
                    tc.tile_pool(name="ps_s", bufs=1, space="PSUM") as ps_s,
                    tc.tile_pool(name="ps_t", bufs=2, space="PSUM") as ps_t,
                    tc.tile_pool(name="ps_a", bufs=2, space="PSUM") as ps_a,
                ):
                    mask_sb = mkp.tile([128, NQB, TKV], F32)
                    nc.sync.dma_start(out=mask_sb[:],
                                      in_=masks.rearrange("b p t -> p b t"))
                    for kvh in range(KVH):
                        for qb in range(NQB):
                            pns = []
                            for j in range(4):
                                h = kvh * 4 + j
                                s_sb = scp.tile([128, TKV], F32, tag=f"s{j}")
                                rs = srp.tile([128, 2], F32, tag=f"rs{j}")
                                for i, (c0, w) in enumerate(SCH):
                                    pss = ps_s.tile([128, w], F32,
                                                    name=f"pssc{i}",
                                                    tag=f"pssc{i}")
                                    nc.tensor.matmul(
                                        pss[:],
                                        qT[:, h, qb * 128:(qb + 1) * 128],
                                        kT[:, kvh, c0:c0 + w])
                                    nc.vector.tensor_add(
                                        s_sb[:, c0:c0 + w], pss[:],
                                        mask_sb[:, qb, c0:c0 + w])
                                nc.scalar.activation(s_sb[:], s_sb[:], AF.Exp,
                                                     accum_out=rs[:, 0:1])
                                nc.vector.reciprocal(rs[:, 1:2], rs[:, 0:1])
                                pn = scp.tile([128, TKV], F32R, tag=f"pn{j}")
                                nc.vector.tensor_scalar_mul(
                                    pn[:], in0=s_sb[:], scalar1=rs[:, 1:2])
                                pns.append(pn)
                            psa = ps_a.tile([128, 512], F32, tag="psa")
                            for kt in range(NKT):
                                ptp = ps_t.tile([128, 512], F32R, tag="ptp")
                                for j in range(4):
                                    nc.tensor.transpose(
                                        ptp[:, j * 128:(j + 1) * 128],
                                        pns[j][:, kt * 128:(kt + 1) * 128],
                                        ident_sb[:])
                                pts = ptsp.tile([128, 512], F32R, tag="pts")
                                nc.scalar.copy(pts[:], ptp[:].bitcast(F32))
                                nc.tensor.matmul(
                                    psa[:],
                                    vN[:, kt, kvh * 128:(kvh + 1) * 128],
                                    pts[:],
                                    start=(kt == 0), stop=(kt == NKT - 1))
                            nc.scalar.copy(
                                attnT[:, kvh * 4:(kvh + 1) * 4,
                                      qb * 128:(qb + 1) * 128],
                                psa[:].rearrange("p (j q) -> p j q", j=4))

                # ---------------- out-proj + residual + LN2 ----------------
                with (
                    tc.tile_pool(name="outp", bufs=1) as outp,
                    tc.tile_pool(name="wop", bufs=3) as wop,
                    tc.tile_pool(name="sq2p", bufs=2) as sq2p,
                    tc.tile_pool(name="ps_o", bufs=2, space="PSUM") as ps_o,
                    tc.tile_pool(name="ps_l2", bufs=1, space="PSUM") as ps_l2,
                ):
                    residT = outp.tile([128, DT, TQ], F32R)
                    h2s = outp.tile([128, DT, TQ], F32R)
                    ps2s = ps_l2.tile([1, TQ], F32, tag="ps2s")
                    ps2q = ps_l2.tile([1, TQ], F32, tag="ps2q")
                    for d2 in range(DT):
                        wo_sb = wop.tile([128, DT, 128], F32R, tag="wo")
                        nc.sync.dma_start(out=wo_sb[:], in_=wo[d2])
                        pso = ps_o.tile([128, TQ], F32, tag="pso")
                        for o in range(DT):
                            nc.tensor.matmul(pso[:], wo_sb[:, o, :],
                                             attnT[:, o, :],
                                             start=(o == 0), stop=(o == DT - 1))
                        nc.vector.tensor_add(residT[:, d2, :], pso[:],
                                             xq_res[:, d2, :])
                        nc.sync.dma_start(out=rest[d2],
                                          in_=residT[:, d2, :].bitcast(F32))
                        sq2 = sq2p.tile([128, TQ], F32R, tag="sq2")
                        nc.scalar.activation(sq2[:],
                                             residT[:, d2, :].bitcast(F32),
                                             AF.Square)
                        nc.tensor.matmul(ps2s[:], ones_sb[:], residT[:, d2, :],
                                         start=(d2 == 0), stop=(d2 == DT - 1))
                        nc.tensor.matmul(ps2q[:], ones_sb[:], sq2[:],
                                         start=(d2 == 0), stop=(d2 == DT - 1))
                    # LN2 rows
                    mu2 = outp.tile([1, TQ], F32)
                    sqm2 = outp.tile([1, TQ], F32)
                    t_r2 = outp.tile([1, TQ], F32)
                    nc.scalar.mul(mu2[:], ps2s[:], 1.0 / D)
                    nc.scalar.mul(sqm2[:], ps2q[:], 1.0 / D)
                    nc.vector.tensor_mul(t_r2[:], mu2[:], mu2[:])
                    nc.vector.tensor_sub(sqm2[:], sqm2[:], t_r2[:])
                    nc.scalar.activation(sqm2[:], sqm2[:], AF.Sqrt,
                                         bias=eps_t[:])
                    nc.vector.reciprocal(sqm2[:], sqm2[:])
                    nc.vector.tensor_mul(t_r2[:], mu2[:], sqm2[:])
                    nc.scalar.mul(t_r2[:], t_r2[:], -1.0)
                    nc.sync.dma_start(out=scratch[2:3, 0:TQ], in_=sqm2[:])
                    nc.sync.dma_start(out=scratch[3:4, 0:TQ], in_=t_r2[:])
                    rstd2_bc = outp.tile([128, TQ], F32)
                    nc.sync.dma_start(out=rstd2_bc[:],
                                      in_=bc_ap(scratch[2:3, 0:TQ], 128, TQ))
                    nmr2_bc = outp.tile([128, TQ], F32)
                    nc.sync.dma_start(out=nmr2_bc[:],
                                      in_=bc_ap(scratch[3:4, 0:TQ], 128, TQ))

                    # ---------------- h2 + router logits ----------------
                    with (
                        tc.tile_pool(name="wrp", bufs=1) as wrp,
                        tc.tile_pool(name="ps_r", bufs=1, space="PSUM") as ps_r,
                    ):
                        wr_sb = wrp.tile([128, DT, 8], F32R)
                        nc.sync.dma_start(out=wr_sb[:], in_=wr[:])
                        psl = ps_r.tile([8, TQ], F32, tag="psl")
                        for d2 in range(DT):
                            nc.vector.tensor_mul(h2s[:, d2, :],
                                                 residT[:, d2, :].bitcast(F32),
                                                 rstd2_bc[:])
                            nc.vector.tensor_add(h2s[:, d2, :],
                                                 h2s[:, d2, :].bitcast(F32),
                                                 nmr2_bc[:])
                            nc.sync.dma_start(out=h2t[d2],
                                              in_=h2s[:, d2, :].bitcast(F32))
                            nc.tensor.matmul(psl[:], wr_sb[:, d2, :],
                                             h2s[:, d2, :],
                                             start=(d2 == 0),
                                             stop=(d2 == DT - 1))
                        lo = outp.tile([8, TQ], F32)
                        nc.scalar.copy(lo[:], psl[:])
                        nc.sync.dma_start(out=logt[:], in_=lo[:])
    nc.compile()
    return nc


# ======================= host-side prep =======================

def core_colmap(r, NB=8, BLK=128):
    """(batch, pos) per column for core r. cols: [own qb0, own qb1, rest]."""
    b = []
    b += [(0, r * BLK + i) for i in range(BLK)]
    b += [(1, (NB - 1 - r) * BLK + i) for i in range(BLK)]
    for j in range(r):
        b += [(0, j * BLK + i) for i in range(BLK)]
    for j in range(NB - 1 - r):
        b += [(1, j * BLK + i) for i in range(BLK)]
    return b


def host_attn_inputs(x, cos, sin, ln1_w, w_qkv, w_out, w_router, ln2_w,
                     n_cores=8):
    """Per-core input maps for build_attn. x [B,S,D]; cos/sin [S,HD]."""
    B, S, Dm = x.shape
    NB, BLK = S // 128, 128
    wqkvT = (w_qkv * ln1_w[None, :]).T.astype(np.float32)      # [D, 3072]
    wqm = wqkvT[:, :NH * HD]                                    # [D, 2048] Q
    wkm = wqkvT[:, NH * HD:NH * HD + 512]                       # [D, 512] K
    wvm = wqkvT[:, NH * HD + 512:]                              # [D, 512] V
    w_outT = w_out.T.astype(np.float32)                         # [O, D]
    sinp = sin.copy()
    sinp[:, :HD // 2] *= -1.0
    scale = np.float32(1.0 / np.sqrt(HD))

    wk_in = np.ascontiguousarray(
        wkm.reshape(DT, 128, KVH, 128).transpose(2, 1, 0, 3))  # [ok, p, d, k]
    wv_in = np.ascontiguousarray(wvm.reshape(DT, 128, 512).transpose(1, 0, 2))
    wq_in = np.ascontiguousarray(
        wqm.reshape(DT, 128, NH, 128).transpose(2, 1, 0, 3))   # [oq, p, d, k]
    wo_in = np.ascontiguousarray(
        w_outT.reshape(DT, 128, DT, 128).transpose(2, 1, 0, 3))  # [d2, p, o, k]
    wr_in = np.ascontiguousarray(
        ((w_router * ln2_w[None, :]).T.astype(np.float32))
        .reshape(DT, 128, 8).transpose(1, 0, 2))               # [p, d, 8]
    wksum = np.ascontiguousarray(wkm.sum(0).reshape(KVH, 128).T)  # [128, KVH]
    wqsum = np.ascontiguousarray(wqm.sum(0).reshape(NH, 128).T)   # [128, NH]
    wvsum = np.ascontiguousarray(wvm.sum(0).reshape(1, 512))
    ident = np.eye(128, dtype=np.float32)
    ones_in = np.ones((128, 1), np.float32)

    maps = []
    for r in range(n_cores):
        cm = core_colmap(r, NB, BLK)
        bs = np.array([c[0] for c in cm])
        ps = np.array([c[1] for c in cm])
        xTc = np.ascontiguousarray(x[bs, ps, :].T)              # [D, TKV]
        ck = np.ascontiguousarray(cos[ps].T)                    # [HD, TKV]
        sk = np.ascontiguousarray(sinp[ps].T)
        cq = np.ascontiguousarray(cos[ps[:TQ]].T) * scale
        sq = np.ascontiguousarray(sinp[ps[:TQ]].T) * scale
        msk = np.full((NQB, 128, TKV), NEG, np.float32)
        for qb in range(NQB):
            qb_b = bs[qb * 128]
            qb_p = ps[qb * 128:(qb + 1) * 128]
            okm = (bs[None, :] == qb_b) & (ps[None, :] <= qb_p[:, None])
            msk[qb][okm] = 0.0
        maps.append({
            "xt": np.ascontiguousarray(xTc.reshape(DT, 128, TKV)),
            "wk": wk_in, "wv": wv_in, "wq": wq_in, "wo": wo_in, "wr": wr_in,
            "wksum": wksum, "wqsum": wqsum, "wvsum": wvsum,
            "cosk": ck, "sink": sk, "cosq": cq, "sinq": sq,
            "masks": msk, "ones": ones_in, "ident": ident,
        })
    return maps


def assemble_attn_outputs(results, n_cores=8, NB=8, BLK=128):
    """results: per-core dicts. Returns h2T_full [D,T], resid_full [D,T],
    logits [T, 8] in (batch, pos) token order."""
    T = 2 * NB * BLK
    h2T = np.zeros((D, T), np.float32)
    rT = np.zeros((D, T), np.float32)
    lg = np.zeros((T, 8), np.float32)
    for r in range(n_cores):
        cm = core_colmap(r, NB, BLK)
        toks = np.array([b * NB * BLK + p for b, p in cm[:TQ]])
        h2T[:, toks] = results[r]["h2t"].reshape(D, TQ)
        rT[:, toks] = results[r]["rest"].reshape(D, TQ)
        lg[toks] = results[r]["logt"].T
    return h2T, rT, lg

# ======================= MoE launch (expert parallel) =======================
MD, MF = 2048, 2048
DT_, FT = MD // 128, MF // 128

def chunks(C):
    # free-dim chunks <=512 (PSUM bank), prefer fewest chunks all >=256
    if C <= 512:
        return [(0, C)]
    if C <= 1024:
        h = (C // 2 + 31) // 32 * 32
        return [(0, h), (h, C - h)]
    return [(0, 512), (512, 512), (1024, C - 1024)]


def build_moe(C, n_cores=8):
    CH = chunks(C)
    nc = bacc.Bacc("TRN2", target_bir_lowering=False, debug=False,
                   num_devices=n_cores)
    xe = nc.dram_tensor("xe", [DT_, 128, C], F32R, kind="ExternalInput").ap()
    wg = nc.dram_tensor("wg", [FT, 128, DT_, 128], F32R, kind="ExternalInput").ap()
    wu = nc.dram_tensor("wu", [FT, 128, DT_, 128], F32R, kind="ExternalInput").ap()
    wd = nc.dram_tensor("wd", [DT_, 128, FT, 128], F32R, kind="ExternalInput").ap()
    wec = nc.dram_tensor("wec", [1, C], F32, kind="ExternalInput").ap()
    ye = nc.dram_tensor("ye", [DT_, 128, C], F32, kind="ExternalOutput").ap()

    with tile.TileContext(nc) as tc:
        with (
            tc.tile_pool(name="res", bufs=1) as res,
            tc.tile_pool(name="wp", bufs=3) as wp,
            tc.tile_pool(name="sg", bufs=3) as sgp,
            tc.tile_pool(name="yo", bufs=3) as yop,
        ):
            xsb = res.tile([128, DT_, C], F32R)
            for d in range(DT_):
                nc.sync.dma_start(out=xsb[:, d, :], in_=xe[d])
            webc = res.tile([128, C], F32)
            nc.sync.dma_start(
                out=webc[:],
                in_=bass.AP(tensor=wec.tensor, offset=wec.offset,
                            ap=[[0, 128], [1, C]]),
            )
            mT = res.tile([128, FT, C], F32R)

            # --- gate/up + silu*u -> mT ---
            with (
                tc.tile_pool(name="psgu", bufs=1, space="PSUM") as psg,
                tc.tile_pool(name="psy", bufs=2, space="PSUM") as psy,
            ):
                for f in range(FT):
                    pgs = [psg.tile([128, w], F32, name=f"pg{ci}", tag=f"pg{ci}")
                           for ci, (_, w) in enumerate(CH)]
                    pus = [psg.tile([128, w], F32, name=f"pu{ci}", tag=f"pu{ci}")
                           for ci, (_, w) in enumerate(CH)]
                    wgt = wp.tile([128, DT_, 128], F32R, tag="wg")
                    nc.sync.dma_start(out=wgt[:], in_=wg[f])
                    wut = wp.tile([128, DT_, 128], F32R, tag="wu")
                    nc.sync.dma_start(out=wut[:], in_=wu[f])
                    for d in range(DT_):
                        for ci, (c0, w) in enumerate(CH):
                            nc.tensor.matmul(pgs[ci][:], wgt[:, d, :],
                                             xsb[:, d, c0:c0 + w],
                                             start=(d == 0), stop=(d == DT_ - 1))
                        for ci, (c0, w) in enumerate(CH):
                            nc.tensor.matmul(pus[ci][:], wut[:, d, :],
                                             xsb[:, d, c0:c0 + w],
                                             start=(d == 0), stop=(d == DT_ - 1))
                    for ci, (c0, w) in enumerate(CH):
                        sg = sgp.tile([128, 512], F32, tag="sg")
                        nc.scalar.activation(sg[:, :w], pgs[ci][:],
                                             mybir.ActivationFunctionType.Silu)
                        nc.vector.tensor_mul(mT[:, f, c0:c0 + w], sg[:, :w],
                                             pus[ci][:])

                # --- down + combine-weight scale -> ye ---
                for d2 in range(DT_):
                    pys = [psy.tile([128, w], F32, name=f"py{ci}", tag=f"py{ci}")
                           for ci, (_, w) in enumerate(CH)]
                    wdt = wp.tile([128, FT, 128], F32R, tag="wd")
                    nc.sync.dma_start(out=wdt[:], in_=wd[d2])
                    for f in range(FT):
                        for ci, (c0, w) in enumerate(CH):
                            nc.tensor.matmul(pys[ci][:], wdt[:, f, :],
                                             mT[:, f, c0:c0 + w],
                                             start=(f == 0), stop=(f == FT - 1))
                    for ci, (c0, w) in enumerate(CH):
                        yt = yop.tile([128, 512], F32, tag="yt")
                        nc.vector.tensor_mul(yt[:, :w], pys[ci][:],
                                             webc[:, c0:c0 + w])
                        nc.sync.dma_start(out=ye[d2, :, c0:c0 + w], in_=yt[:, :w])
    nc.compile()
    return nc


def host_moe_inputs(h2T_full, assign, aw, C, w_gate_f, w_up_f, w_down):
    """Build per-core input maps. h2T_full [D, T]; assign/aw lists per expert."""
    E = len(assign)
    maps = []
    for e in range(E):
        n = len(assign[e])
        assert n <= C, f"expert {e} count {n} > capacity {C}"
        xeT = np.zeros((MD, C), np.float32)
        xeT[:, :n] = h2T_full[:, assign[e]]
        wec = np.zeros((1, C), np.float32)
        wec[0, :n] = aw[e]
        maps.append({
            "xe": np.ascontiguousarray(xeT.reshape(DT_, 128, C)),
            "wg": np.ascontiguousarray(
                w_gate_f[e].reshape(DT_, 128, FT, 128).transpose(2, 1, 0, 3)),
            "wu": np.ascontiguousarray(
                w_up_f[e].reshape(DT_, 128, FT, 128).transpose(2, 1, 0, 3)),
            "wd": np.ascontiguousarray(
                w_down[e].reshape(FT, 128, DT_, 128).transpose(2, 1, 0, 3)),
            "wec": wec,
        })
    return maps


# ======================= top-level kernel =======================
E, K_TOP = 8, 2
_cache = {}


def _routing(logits):
    lm = logits.max(1, keepdims=True)
    p = np.exp(logits - lm)
    p /= p.sum(1, keepdims=True)
    top_e = np.argsort(-p, 1)[:, :K_TOP]
    top_w = np.take_along_axis(p, top_e, 1)
    top_w = top_w / np.abs(top_w).sum(1, keepdims=True)
    flat_e = top_e.ravel()
    flat_t = np.repeat(np.arange(logits.shape[0]), K_TOP)
    flat_w = top_w.ravel()
    assign = [flat_t[flat_e == e] for e in range(E)]
    aw = [flat_w[flat_e == e] for e in range(E)]
    return assign, aw


def kernel(hidden_states, cos, sin, ln1_w, ln2_w, w_qkv, w_out,
           w_router, w_gate, w_up, w_down):
    hidden_states = np.asarray(hidden_states, np.float32)
    cos = np.asarray(cos, np.float32)
    sin = np.asarray(sin, np.float32)
    ln1_w = np.asarray(ln1_w, np.float32)
    ln2_w = np.asarray(ln2_w, np.float32)
    w_qkv = np.asarray(w_qkv, np.float32)
    w_out = np.asarray(w_out, np.float32)
    w_router = np.asarray(w_router, np.float32)
    w_gate = np.asarray(w_gate, np.float32)
    w_up = np.asarray(w_up, np.float32)
    w_down = np.asarray(w_down, np.float32)
    B, S, Dm = hidden_states.shape

    if "attn" not in _cache:
        _cache["attn"] = build_attn()
    maps = host_attn_inputs(hidden_states, cos, sin, ln1_w, w_qkv, w_out,
                            w_router, ln2_w)
    res1 = run_bass_kernel_spmd(_cache["attn"], maps, list(range(8)))
    h2T, rT, lg = assemble_attn_outputs(res1.results)

    assign, aw = _routing(lg)
    counts = [len(a) for a in assign]
    C = max(256, (max(counts) + 63) // 64 * 64)

    if ("moe", C) not in _cache:
        _cache[("moe", C)] = build_moe(C)
    w_gate_f = w_gate * ln2_w[None, :, None]
    w_up_f = w_up * ln2_w[None, :, None]
    maps2 = host_moe_inputs(h2T, assign, aw, C, w_gate_f, w_up_f, w_down)
    res2 = run_bass_kernel_spmd(_cache[("moe", C)], maps2, list(range(8)))

    T = B * S
    out_full = np.zeros((T, MD), np.float32)
    for e in range(E):
        ye = res2.results[e]["ye"].reshape(MD, C)
        n = counts[e]
        out_full[assign[e]] += ye[:, :n].T

    out = out_full.reshape(B, S, Dm)
    residual = rT.T.reshape(B, S, Dm)
    return out, residual



# revision 9
# speedup vs baseline: 1.7240x; 1.7240x over previous
"""Self-contained Trainium2 Bass kernel for nn_DbrxBlock_40492951667588.

DBRX block: LN1 -> GQA attention (RoPE, causal) -> residual+LN2 -> top-2/8 MoE.
8 NeuronCores, two SPMD launches, all matmuls in bf16:
  launch 1: attention sharded by (batch, kv-head) — core r owns batch r//4,
            kv-head r%4 (4 q heads). Scores computed transposed [k, q] so
            PV needs no PE transposes; causal block-skipping; per-column
            softmax normalization via a K=1 broadcast matmul.
  host:     LN1 (pre-launch), partial-sum reduce + residual + LN2 + router
            softmax/top-2 + capacity-padded expert dispatch (between launches).
  launch 2: expert-parallel MoE (core e owns expert e).
"""
import numpy as np
import ml_dtypes
import concourse.bacc as bacc
import concourse.bass as bass
import concourse.mybir as mybir
import concourse.tile as tile
from concourse.bass_utils import run_bass_kernel_spmd

F32 = mybir.dt.float32
BF16 = mybir.dt.bfloat16
AF = mybir.ActivationFunctionType
BF = ml_dtypes.bfloat16

B, S, D = 2, 1024, 2048
DT = D // 128            # 16 d-tiles
NH, KVH, HD = 16, 4, 128
GQ = NH // KVH           # 4 q heads per kv head
NKT = S // 128           # 8 token tiles
EPS = 1e-5
NEG = -30000.0
E, K_TOP = 8, 2
MD, MF = 2048, 2048
DT_, FT = MD // 128, MF // 128


# ======================= attention launch =======================

def build_attn(n_cores=8):
    nc = bacc.Bacc("TRN2", target_bir_lowering=False, debug=False,
                   num_devices=n_cores)
    xln = nc.dram_tensor("xln", [DT, 128, S], BF16, kind="ExternalInput").ap()
    wk1 = nc.dram_tensor("wk1", [128, DT, 128], BF16, kind="ExternalInput").ap()
    wv1 = nc.dram_tensor("wv1", [128, DT, 128], BF16, kind="ExternalInput").ap()
    wq4 = nc.dram_tensor("wq4", [128, GQ * DT, 128], BF16,
                         kind="ExternalInput").ap()
    wo4 = nc.dram_tensor("wo4", [128, GQ * DT, 128], BF16,
                         kind="ExternalInput").ap()
    cosk = nc.dram_tensor("cosk", [128, S], F32, kind="ExternalInput").ap()
    sink = nc.dram_tensor("sink", [128, S], F32, kind="ExternalInput").ap()
    maskd = nc.dram_tensor("maskd", [128, 128], F32, kind="ExternalInput").ap()
    onesc = nc.dram_tensor("onesc", [128, 1], BF16, kind="ExternalInput").ap()
    onesr = nc.dram_tensor("onesr", [1, 128], F32, kind="ExternalInput").ap()
    po = nc.dram_tensor("po", [DT, 128, S], BF16, kind="ExternalOutput").ap()

    HQ = [(0, 512), (512, 512)]  # S-span psum chunks

    with tile.TileContext(nc) as tc:
        with (
            tc.tile_pool(name="cst", bufs=1) as cst,
            tc.tile_pool(name="big", bufs=1) as big,
        ):
            onesc_sb = cst.tile([128, 1], BF16)
            nc.sync.dma_start(out=onesc_sb[:], in_=onesc[:])
            onesr_sb = cst.tile([1, 128], F32)
            nc.sync.dma_start(out=onesr_sb[:], in_=onesr[:])
            maskd_sb = cst.tile([128, 128], F32)
            nc.sync.dma_start(out=maskd_sb[:], in_=maskd[:])
            cos_sb = cst.tile([128, S], F32)
            nc.sync.dma_start(out=cos_sb[:], in_=cosk[:])
            sin_sb = cst.tile([128, S], F32)
            nc.sync.dma_start(out=sin_sb[:], in_=sink[:])

            xln_sb = big.tile([128, DT, S], BF16)
            for d in range(DT):
                nc.sync.dma_start(out=xln_sb[:, d, :], in_=xln[d])

            kT = big.tile([128, S], BF16)          # [HD, k]
            vN = big.tile([128, NKT, HD], BF16)    # [tok, kt, hd]
            qT = big.tile([128, GQ, S], BF16)      # [HD, h, q]
            attnT = big.tile([128, GQ, S], BF16)   # [hd, h, q]
            # P tiles, one per kt: [k-tok, h, q span]
            P_kt = [big.tile([128, GQ, S - kt * 128], BF16, name=f"P{kt}")
                    for kt in range(NKT)]

            # ---------------- K proj + rope ----------------
            with (
                tc.tile_pool(name="wkp", bufs=1) as wkp,
                tc.tile_pool(name="rp", bufs=1) as rp,
                tc.tile_pool(name="ps_k", bufs=2, space="PSUM") as ps_k,
            ):
                wk_sb = wkp.tile([128, DT, 128], BF16)
                nc.sync.dma_start(out=wk_sb[:], in_=wk1[:])
                ktmp = rp.tile([128, S], F32)
                krot = rp.tile([128, S], F32)
                tmpa = rp.tile([128, S], F32)
                tmpb = rp.tile([128, S], F32)
                for c0, w in HQ:
                    psk = ps_k.tile([128, 512], F32, tag="psk")
                    for d in range(DT):
                        nc.tensor.matmul(psk[:], wk_sb[:, d, :],
                                         xln_sb[:, d, c0:c0 + w],
                                         start=(d == 0), stop=(d == DT - 1))
                    nc.scalar.copy(ktmp[:, c0:c0 + w], psk[:])
                nc.sync.dma_start(out=krot[0:64, :], in_=ktmp[64:128, :])
                nc.sync.dma_start(out=krot[64:128, :], in_=ktmp[0:64, :])
                nc.vector.tensor_mul(tmpa[:], ktmp[:], cos_sb[:])
                nc.vector.tensor_mul(tmpb[:], krot[:], sin_sb[:])
                nc.vector.tensor_add(kT[:], tmpa[:], tmpb[:])

                # ---------------- Q proj + rope (reuse rp tiles) ----------
                wq_sb = wkp.tile([128, GQ * DT, 128], BF16)
                nc.sync.dma_start(out=wq_sb[:], in_=wq4[:])
                for j in range(GQ):
                    qtmp = rp.tile([128, S], F32, tag=f"qt{j % 2}",
                                   name=f"qtmp{j}")
                    qrot = rp.tile([128, S], F32, tag=f"qr{j % 2}",
                                   name=f"qrot{j}")
                    for c0, w in HQ:
                        psq = ps_k.tile([128, 512], F32, tag="psq")
                        for d in range(DT):
                            nc.tensor.matmul(psq[:], wq_sb[:, j * DT + d, :],
                                             xln_sb[:, d, c0:c0 + w],
                                             start=(d == 0), stop=(d == DT - 1))
                        nc.scalar.copy(qtmp[:, c0:c0 + w], psq[:])
                    nc.sync.dma_start(out=qrot[0:64, :], in_=qtmp[64:128, :])
                    nc.sync.dma_start(out=qrot[64:128, :], in_=qtmp[0:64, :])
                    nc.vector.tensor_mul(qtmp[:], qtmp[:], cos_sb[:])
                    nc.vector.tensor_mul(qrot[:], qrot[:], sin_sb[:])
                    nc.vector.tensor_add(qT[:, j, :], qtmp[:], qrot[:])

                # ---------------- V proj (t-major) ----------------
                wv_sb = wkp.tile([128, DT, 128], BF16)
                nc.sync.dma_start(out=wv_sb[:], in_=wv1[:])
                for kt in range(NKT):
                    psv = ps_k.tile([128, 128], F32, tag="psv")
                    for d in range(DT):
                        nc.tensor.matmul(psv[:],
                                         xln_sb[:, d, kt * 128:(kt + 1) * 128],
                                         wv_sb[:, d, :],
                                         start=(d == 0), stop=(d == DT - 1))
                    nc.scalar.copy(vN[:, kt, :], psv[:])

            # ---------------- scores^T + exp ----------------
            with (
                tc.tile_pool(name="ps_s", bufs=3, space="PSUM") as ps_s,
            ):
                for kt in range(NKT):
                    span = S - kt * 128
                    chunks = [(c, min(512, span - c))
                              for c in range(0, span, 512)]
                    for j in range(GQ):
                        for c0, w in chunks:
                            psS = ps_s.tile([128, 512], F32, tag="psS")
                            nc.tensor.matmul(
                                psS[:, :w],
                                kT[:, kt * 128:(kt + 1) * 128],
                                qT[:, j, kt * 128 + c0:kt * 128 + c0 + w])
                            if c0 == 0:  # diagonal block: causal mask
                                nc.vector.tensor_add(psS[:, 0:128],
                                                     psS[:, 0:128],
                                                     maskd_sb[:])
                            nc.scalar.activation(P_kt[kt][:, j, c0:c0 + w],
                                                 psS[:, :w], AF.Exp)

                # ---------------- sums + PV + normalize ----------------
                with (
                    tc.tile_pool(name="nrm", bufs=2) as nrm,
                    tc.tile_pool(name="ps_sum", bufs=1, space="PSUM") as ps_sum,
                    tc.tile_pool(name="ps_pv", bufs=2, space="PSUM") as ps_pv,
                ):
                    for qt in range(NKT):
                        sums_ps = ps_sum.tile([1, GQ * 128], F32, tag="sums")
                        pv_ps = ps_pv.tile([128, GQ * 128], F32, tag="pv")
                        for kt in range(qt + 1):
                            qoff = (qt - kt) * 128
                            nc.tensor.matmul(sums_ps[:], onesc_sb[:],
                                             P_kt[kt][:, :, qoff:qoff + 128],
                                             start=(kt == 0), stop=(kt == qt))
                        for kt in range(qt + 1):
                            qoff = (qt - kt) * 128
                            nc.tensor.matmul(pv_ps[:], vN[:, kt, :],
                                             P_kt[kt][:, :, qoff:qoff + 128],
                                             start=(kt == 0), stop=(kt == qt))
                        recip_sb = nrm.tile([1, GQ * 128], F32, tag="recip")
                        nc.vector.reciprocal(recip_sb[:], sums_ps[:])
                        rbc_ps = ps_sum.tile([128, GQ * 128], F32, tag="rbc")
                        nc.tensor.matmul(rbc_ps[:], onesr_sb[:], recip_sb[:])
                        rbc_sb = nrm.tile([128, GQ * 128], F32, tag="rbcs")
                        nc.vector.tensor_copy(rbc_sb[:], rbc_ps[:])
                        nc.vector.tensor_mul(
                            attnT[:, :, qt * 128:(qt + 1) * 128],
                            pv_ps[:].rearrange("p (j q) -> p j q", j=GQ),
                            rbc_sb[:].rearrange("p (j q) -> p j q", j=GQ))

            # ---------------- out proj (partial over 4 heads) ----------
            with (
                tc.tile_pool(name="wop", bufs=1) as wop,
                tc.tile_pool(name="pop", bufs=3) as pop,
                tc.tile_pool(name="ps_o", bufs=2, space="PSUM") as ps_o,
            ):
                wo_sb = wop.tile([128, GQ * DT, 128], BF16)
                nc.sync.dma_start(out=wo_sb[:], in_=wo4[:])
                for d2 in range(DT):
                    for c0, w in HQ:
                        pso = ps_o.tile([128, 512], F32, tag="pso")
                        for j in range(GQ):
                            nc.tensor.matmul(pso[:], wo_sb[:, j * DT + d2, :],
                                             attnT[:, j, c0:c0 + w],
                                             start=(j == 0), stop=(j == GQ - 1))
                        po_t = pop.tile([128, 512], BF16, tag="pot")
                        nc.scalar.copy(po_t[:], pso[:])
                        nc.sync.dma_start(out=po[d2, :, c0:c0 + w], in_=po_t[:])
    nc.compile()
    return nc


def _ln(x):
    mu = x.mean(-1, keepdims=True)
    var = x.var(-1, keepdims=True)
    return (x - mu) / np.sqrt(var + EPS)


def host_attn_inputs(x, cos, sin, ln1_w, w_qkv, w_out, n_cores=8):
    """Per-core input maps for build_attn. x [B,S,D] f32; cos/sin [S,HD]."""
    xln = (_ln(x) * ln1_w[None, None, :]).astype(np.float32)  # [B,S,D]
    wqkvT = w_qkv.T.astype(np.float32)                        # [D, 3072]
    scale = np.float32(1.0 / np.sqrt(HD))
    wq_all = wqkvT[:, :NH * HD] * scale
    wk_all = wqkvT[:, NH * HD:(NH + KVH) * HD]
    wv_all = wqkvT[:, (NH + KVH) * HD:]
    w_outT = w_out.T.astype(np.float32)                       # [O, D]
    sinp = sin.copy()
    sinp[:, :HD // 2] *= -1.0
    cosT = np.ascontiguousarray(cos.T).astype(np.float32)     # [HD, S]
    sinT = np.ascontiguousarray(sinp.T).astype(np.float32)

    kk = np.arange(128)[:, None]
    qq = np.arange(128)[None, :]
    maskd = np.where(kk <= qq, 0.0, NEG).astype(np.float32)
    onesc = np.ones((128, 1), BF)
    onesr = np.ones((1, 128), np.float32)

    xln_b = [np.ascontiguousarray(xln[b].T).astype(BF).reshape(DT, 128, S)
             for b in range(B)]
    maps = []
    for r in range(n_cores):
        b, g = divmod(r, KVH)
        wk_in = np.ascontiguousarray(
            wk_all[:, g * 128:(g + 1) * 128]
            .reshape(DT, 128, 128).transpose(1, 0, 2)).astype(BF)
        wv_in = np.ascontiguousarray(
            wv_all[:, g * 128:(g + 1) * 128]
            .reshape(DT, 128, 128).transpose(1, 0, 2)).astype(BF)
        wq_in = np.ascontiguousarray(
            wq_all[:, g * GQ * HD:(g + 1) * GQ * HD]
            .reshape(DT, 128, GQ, 128).transpose(1, 2, 0, 3)
            .reshape(128, GQ * DT, 128)).astype(BF)
        wo_in = np.ascontiguousarray(
            w_outT[g * GQ * HD:(g + 1) * GQ * HD]
            .reshape(GQ, 128, DT, 128).transpose(1, 0, 2, 3)
            .reshape(128, GQ * DT, 128)).astype(BF)
        maps.append({
            "xln": xln_b[b], "wk1": wk_in, "wv1": wv_in, "wq4": wq_in,
            "wo4": wo_in, "cosk": cosT, "sink": sinT, "maskd": maskd,
            "onesc": onesc, "onesr": onesr,
        })
    return maps


def assemble_attn_outputs(results, x, n_cores=8):
    """Sum per-core partial out-projections, add residual. Returns resid
    [B,S,D] f32."""
    attn = np.zeros((B, D, S), np.float32)
    for r in range(n_cores):
        b = r // KVH
        attn[b] += results[r]["po"].reshape(D, S).astype(np.float32)
    resid = x + attn.transpose(0, 2, 1)
    return resid


# ======================= MoE launch (expert parallel) =======================

def chunks(C):
    # free-dim chunks <=512 (PSUM bank), prefer fewest chunks all >=256
    if C <= 512:
        return [(0, C)]
    if C <= 1024:
        h = (C // 2 + 31) // 32 * 32
        return [(0, h), (h, C - h)]
    return [(0, 512), (512, 512), (1024, C - 1024)]


def build_moe(C, n_cores=8, _act=None):
    act = AF.Silu if _act is None else _act
    CH = chunks(C)
    nc = bacc.Bacc("TRN2", target_bir_lowering=False, debug=False,
                   num_devices=n_cores)
    xe = nc.dram_tensor("xe", [DT_, 128, C], BF16, kind="ExternalInput").ap()
    wg = nc.dram_tensor("wg", [FT, 128, DT_, 128], BF16,
                        kind="ExternalInput").ap()
    wu = nc.dram_tensor("wu", [FT, 128, DT_, 128], BF16,
                        kind="ExternalInput").ap()
    wd = nc.dram_tensor("wd", [DT_, 128, FT, 128], BF16,
                        kind="ExternalInput").ap()
    wec = nc.dram_tensor("wec", [1, C], F32, kind="ExternalInput").ap()
    ye = nc.dram_tensor("ye", [DT_, 128, C], BF16, kind="ExternalOutput").ap()

    with tile.TileContext(nc) as tc:
        with (
            tc.tile_pool(name="res", bufs=1) as res,
            tc.tile_pool(name="wp", bufs=3) as wp,
            tc.tile_pool(name="sg", bufs=3) as sgp,
            tc.tile_pool(name="yo", bufs=3) as yop,
        ):
            xsb = res.tile([128, DT_, C], BF16)
            for d in range(DT_):
                nc.sync.dma_start(out=xsb[:, d, :], in_=xe[d])
            webc = res.tile([128, C], F32)
            nc.sync.dma_start(
                out=webc[:],
                in_=bass.AP(tensor=wec.tensor, offset=wec.offset,
                            ap=[[0, 128], [1, C]]),
            )
            mT = res.tile([128, FT, C], BF16)

            # --- gate/up + silu*u -> mT ---
            with (
                tc.tile_pool(name="psgu", bufs=1, space="PSUM") as psg,
                tc.tile_pool(name="psy", bufs=2, space="PSUM") as psy,
            ):
                for f in range(FT):
                    pgs = [psg.tile([128, w], F32, name=f"pg{ci}", tag=f"pg{ci}")
                           for ci, (_, w) in enumerate(CH)]
                    pus = [psg.tile([128, w], F32, name=f"pu{ci}", tag=f"pu{ci}")
                           for ci, (_, w) in enumerate(CH)]
                    wgt = wp.tile([128, DT_, 128], BF16, tag="wg")
                    nc.sync.dma_start(out=wgt[:], in_=wg[f])
                    wut = wp.tile([128, DT_, 128], BF16, tag="wu")
                    nc.sync.dma_start(out=wut[:], in_=wu[f])
                    for d in range(DT_):
                        for ci, (c0, w) in enumerate(CH):
                            nc.tensor.matmul(pgs[ci][:], wgt[:, d, :],
                                             xsb[:, d, c0:c0 + w],
                                             start=(d == 0), stop=(d == DT_ - 1))
                        for ci, (c0, w) in enumerate(CH):
                            nc.tensor.matmul(pus[ci][:], wut[:, d, :],
                                             xsb[:, d, c0:c0 + w],
                                             start=(d == 0), stop=(d == DT_ - 1))
                    for ci, (c0, w) in enumerate(CH):
                        sg = sgp.tile([128, 512], F32, tag="sg")
                        nc.scalar.activation(sg[:, :w], pgs[ci][:], act)
                        nc.vector.tensor_mul(mT[:, f, c0:c0 + w], sg[:, :w],
                                             pus[ci][:])

                # --- down + combine-weight scale -> ye ---
                for d2 in range(DT_):
                    pys = [psy.tile([128, w], F32, name=f"py{ci}", tag=f"py{ci}")
                           for ci, (_, w) in enumerate(CH)]
                    wdt = wp.tile([128, FT, 128], BF16, tag="wd")
                    nc.sync.dma_start(out=wdt[:], in_=wd[d2])
                    for f in range(FT):
                        for ci, (c0, w) in enumerate(CH):
                            nc.tensor.matmul(pys[ci][:], wdt[:, f, :],
                                             mT[:, f, c0:c0 + w],
                                             start=(f == 0), stop=(f == FT - 1))
                    for ci, (c0, w) in enumerate(CH):
                        yt = yop.tile([128, 512], BF16, tag="yt")
                        nc.vector.tensor_mul(yt[:, :w], pys[ci][:],
                                             webc[:, c0:c0 + w])
                        nc.sync.dma_start(out=ye[d2, :, c0:c0 + w], in_=yt[:, :w])
    nc.compile()
    return nc


def host_moe_inputs(h2T_full, assign, aw, C, w_gate, w_up, w_down):
    """Per-core input maps. h2T_full [D, T] f32; assign/aw lists per expert."""
    maps = []
    for e in range(E):
        n = len(assign[e])
        assert n <= C, f"expert {e} count {n} > capacity {C}"
        xeT = np.zeros((MD, C), BF)
        xeT[:, :n] = h2T_full[:, assign[e]].astype(BF)
        wec = np.zeros((1, C), np.float32)
        wec[0, :n] = aw[e]
        maps.append({
            "xe": np.ascontiguousarray(xeT.reshape(DT_, 128, C)),
            "wg": np.ascontiguousarray(
                w_gate[e].reshape(DT_, 128, FT, 128)
                .transpose(2, 1, 0, 3)).astype(BF),
            "wu": np.ascontiguousarray(
                w_up[e].reshape(DT_, 128, FT, 128)
                .transpose(2, 1, 0, 3)).astype(BF),
            "wd": np.ascontiguousarray(
                w_down[e].reshape(FT, 128, DT_, 128)
                .transpose(2, 1, 0, 3)).astype(BF),
            "wec": wec,
        })
    return maps


# ======================= top-level kernel =======================
_cache = {}


def _routing(logits):
    lm = logits.max(1, keepdims=True)
    p = np.exp(logits - lm)
    p /= p.sum(1, keepdims=True)
    top_e = np.argsort(-p, 1)[:, :K_TOP]
    top_w = np.take_along_axis(p, top_e, 1)
    top_w = top_w / np.abs(top_w).sum(1, keepdims=True)
    flat_e = top_e.ravel()
    flat_t = np.repeat(np.arange(logits.shape[0]), K_TOP)
    flat_w = top_w.ravel()
    assign = [flat_t[flat_e == e] for e in range(E)]
    aw = [flat_w[flat_e == e] for e in range(E)]
    return assign, aw


def kernel(hidden_states, cos, sin, ln1_w, ln2_w, w_qkv, w_out,
           w_router, w_gate, w_up, w_down):
    hidden_states = np.asarray(hidden_states, np.float32)
    cos = np.asarray(cos, np.float32)
    sin = np.asarray(sin, np.float32)
    ln1_w = np.asarray(ln1_w, np.float32)
    ln2_w = np.asarray(ln2_w, np.float32)
    w_qkv = np.asarray(w_qkv, np.float32)
    w_out = np.asarray(w_out, np.float32)
    w_router = np.asarray(w_router, np.float32)
    w_gate = np.asarray(w_gate, np.float32)
    w_up = np.asarray(w_up, np.float32)
    w_down = np.asarray(w_down, np.float32)

    if "attn" not in _cache:
        _cache["attn"] = build_attn()
    maps = host_attn_inputs(hidden_states, cos, sin, ln1_w, w_qkv, w_out)
    res1 = run_bass_kernel_spmd(_cache["attn"], maps, list(range(8)))
    resid = assemble_attn_outputs(res1.results, hidden_states)

    h2 = (_ln(resid) * ln2_w[None, None, :]).reshape(-1, D)   # [T, D]
    logits = h2 @ w_router.T
    assign, aw = _routing(logits)
    counts = [len(a) for a in assign]
    C = max(256, (max(counts) + 63) // 64 * 64)

    if ("moe", C) not in _cache:
        _cache[("moe", C)] = build_moe(C)
    h2T = np.ascontiguousarray(h2.T)                          # [D, T]
    maps2 = host_moe_inputs(h2T, assign, aw, C, w_gate, w_up, w_down)
    res2 = run_bass_kernel_spmd(_cache[("moe", C)], maps2, list(range(8)))

    T = B * S
    out_full = np.zeros((T, MD), np.float32)
    for e in range(E):
        ye = res2.results[e]["ye"].reshape(MD, C).astype(np.float32)
        n = counts[e]
        out_full[assign[e]] += ye[:, :n].T

    out = out_full.reshape(B, S, D)
    return out, resid
